# revision 7
# baseline (speedup 1.0000x reference)
"""Trainium2 Bass kernel for the NeuralALU32 problem.

The reference module implements exact 32-bit integer addition through
one-hot byte encodings, lookup-table matmuls and sharpness-100 softmaxes.
In float32 the softmaxes collapse to a closed form: for every (token, byte)
the output row over the 256 byte values is

    out[x] = uh[hi(x)] * ul[lo(x)]

where uh/ul are 16-vectors equal to 1.0 at the result nibble of the exact
integer sum (with ripple carry across the 4 bytes) and exp(-50) elsewhere.
All cross terms land at exp(-100) (f32 denormal, and 0 on hardware with FTZ),
matching the reference to absolute error < 2e-27.

The kernel computes the carry chain in f32 (exact for byte values), builds
the nibble one-hot-ish vectors with iota + is_equal, and expands each
token's 1024-float output row with a single broadcast tensor_tensor
multiply. It is output-bandwidth bound: each core writes 32 MiB.

Sharding: pure data parallel over the batch dim, 8192 tokens per core.
"""

import numpy as np

N_CORES = 8
B_FULL = 65536
B_SHARD = B_FULL // N_CORES      # 8192 tokens per core
P = 128                          # SBUF partitions
NPT = B_SHARD // P               # tokens per partition (64)
TPB = 4                          # tokens per partition per output tile
NTILES = NPT // TPB              # output tiles per core (16)

E50 = float(np.float32(np.exp(np.float64(-50.0))))   # 1.9287499e-22


def _emit(tc, nc, a_ap, b_ap, out_ap, npt=NPT, tpb=TPB):
    """Emit the per-core Tile program.

    a_ap, b_ap: [P*npt, 4] int32 DRAM.  out_ap: [P*npt, 1024] f32 DRAM.
    Token t = p*npt + n lives on partition p, free slot n.
    """
    from contextlib import ExitStack
    import concourse.mybir as mybir

    ntiles = npt // tpb
    nc4 = npt * 4                    # free size of per-token-byte tensors
    f32 = mybir.dt.float32
    i32 = mybir.dt.int32
    Alu = mybir.AluOpType

    with ExitStack() as ctx:
        const = ctx.enter_context(tc.tile_pool(name="const", bufs=1))
        pre = ctx.enter_context(tc.tile_pool(name="pre", bufs=1))
        uvs = ctx.enter_context(tc.tile_pool(name="uvs", bufs=3))
        outs = ctx.enter_context(tc.tile_pool(name="outs", bufs=4))

        # --- constants: J[p, m*16+j] = j  (j pattern repeating every 16)
        ji = const.tile([P, 256], i32, tag="ji")
        nc.gpsimd.iota(ji[:], pattern=[[0, 16], [1, 16]], base=0,
                       channel_multiplier=0)
        e50b = const.tile([P, 1], f32, tag="e50b")   # ACT bias vector
        nc.vector.memset(e50b[:], E50)

        # --- load inputs: partition p holds tokens p*npt .. p*npt+npt-1
        ai = pre.tile([P, nc4], i32, tag="ai")
        bi = pre.tile([P, nc4], i32, tag="bi")
        nc.sync.dma_start(ai[:], a_ap.rearrange("(p n) c -> p (n c)", p=P))
        nc.sync.dma_start(bi[:], b_ap.rearrange("(p n) c -> p (n c)", p=P))

        # --- s[p, n, i] = a byte + b byte, ripple carry in int32
        s = pre.tile([P, nc4], i32, tag="s")
        nc.vector.tensor_add(s[:], ai[:], bi[:])
        s3 = s[:].rearrange("p (n c) -> p n c", c=4)

        r = pre.tile([P, nc4], i32, tag="r")      # result bytes
        r3 = r[:].rearrange("p (n c) -> p n c", c=4)
        t = pre.tile([P, npt], i32, tag="t")      # byte sum incl. carry-in
        c = pre.tile([P, npt], i32, tag="c")      # carry out (0/1)
        nc.vector.tensor_copy(t[:], s3[:, :, 0])
        for i in range(4):
            nc.vector.tensor_scalar(r3[:, :, i], t[:], 255, None,
                                    Alu.bitwise_and)
            if i < 3:
                nc.vector.tensor_scalar(c[:], t[:], 8, None,
                                        Alu.logical_shift_right)
                nc.vector.tensor_add(t[:], s3[:, :, i + 1], c[:])

        # --- nibbles: lo = r & 15, hi = r >> 4
        lo = pre.tile([P, nc4], i32, tag="lo")
        hi = pre.tile([P, nc4], i32, tag="hi")
        nc.vector.tensor_scalar(lo[:], r[:], 15, None, Alu.bitwise_and)
        nc.vector.tensor_scalar(hi[:], r[:], 4, None,
                                Alu.logical_shift_right)

        # --- per output tile: tpb tokens per partition -> m = tpb*4 combos
        out_v = out_ap.rearrange("(p n) f -> p n f", p=P)
        m = tpb * 4                  # (token, byte) combos per tile (16)
        for ti in range(ntiles):
            hs = hi[:, ti * m:(ti + 1) * m]
            ls = lo[:, ti * m:(ti + 1) * m]

            uh = uvs.tile([P, m * 16], f32, tag="uh")
            ul = uvs.tile([P, m * 16], f32, tag="ul")
            nc.vector.tensor_tensor(
                uh[:].rearrange("p (m j) -> p m j", m=m),
                ji[:, :m * 16].rearrange("p (m j) -> p m j", m=m),
                hs.to_broadcast((P, m, 16)), Alu.is_equal)
            nc.vector.tensor_tensor(
                ul[:].rearrange("p (m k) -> p m k", m=m),
                ji[:, :m * 16].rearrange("p (m k) -> p m k", m=m),
                ls.to_broadcast((P, m, 16)), Alu.is_equal)
            uhe = uvs.tile([P, m * 16], f32, tag="uhe")
            ule = uvs.tile([P, m * 16], f32, tag="ule")
            nc.scalar.add(uhe[:], uh[:], e50b[:])
            nc.scalar.add(ule[:], ul[:], e50b[:])

            ot = outs.tile([P, m * 256], f32, tag="ot")
            nc.vector.tensor_tensor(
                ot[:].rearrange("p (m j k) -> p m j k", m=m, j=16),
                uhe[:].rearrange("p (m j) -> p m j", m=m).to_broadcast(
                    (P, m, 16, 16)),
                ule[:].rearrange("p (m k) -> p m k", m=m).unsqueeze(
                    2).broadcast_to((P, m, 16, 16)),
                Alu.mult)
            nc.sync.dma_start(out_v[:, ti * tpb:(ti + 1) * tpb, :],
                              ot[:].rearrange("p (n f) -> p n f", n=tpb))


def build_nc(b_shard=B_SHARD, tpb=TPB):
    import concourse.tile as tile
    from concourse import bacc, mybir

    npt = b_shard // P
    nc = bacc.Bacc("TRN2", target_bir_lowering=False, debug=False,
                   num_devices=N_CORES)
    a = nc.dram_tensor("a_idx", [b_shard, 4], mybir.dt.int32,
                       kind="ExternalInput")
    b = nc.dram_tensor("b_idx", [b_shard, 4], mybir.dt.int32,
                       kind="ExternalInput")
    out = nc.dram_tensor("out", [b_shard, 1024], mybir.dt.float32,
                         kind="ExternalOutput")
    with tile.TileContext(nc) as tc:
        _emit(tc, nc, a.ap(), b.ap(), out.ap(), npt=npt, tpb=tpb)
    nc.compile()
    return nc


_NC_CACHE = {}
LAST_RESULTS = None   # BassKernelResults of the most recent kernel() call


def kernel(**inputs):
    a_idx = np.ascontiguousarray(inputs["a_idx"], dtype=np.int32)
    b_idx = np.ascontiguousarray(inputs["b_idx"], dtype=np.int32)
    assert a_idx.shape == (B_FULL, 4) and b_idx.shape == (B_FULL, 4)

    from concourse.bass_utils import run_bass_kernel_spmd

    if "nc" not in _NC_CACHE:
        _NC_CACHE["nc"] = build_nc()
    nc = _NC_CACHE["nc"]

    in_maps = [
        {"a_idx": a_idx[i * B_SHARD:(i + 1) * B_SHARD],
         "b_idx": b_idx[i * B_SHARD:(i + 1) * B_SHARD]}
        for i in range(N_CORES)
    ]
    res = run_bass_kernel_spmd(nc, in_maps, list(range(N_CORES)))
    global LAST_RESULTS
    LAST_RESULTS = res
    out = np.concatenate(
        [r["out"].reshape(B_SHARD, 4, 256) for r in res.results], axis=0)
    return out


# revision 11
# speedup vs baseline: 1.0630x; 1.0630x over previous
"""Trainium2 Bass kernel for the NeuralALU32 problem.

The reference module implements exact 32-bit integer addition through
one-hot byte encodings, lookup-table matmuls and sharpness-100 softmaxes.
In float32 the softmaxes collapse to a closed form: for every (token, byte)
the output row over the 256 byte values is

    out[x] = uh[hi(x)] * ul[lo(x)]

where uh/ul are 16-vectors equal to 1.0 at the result nibble of the exact
integer sum (with ripple carry across the 4 bytes) and exp(-50) elsewhere.
All cross terms land at exp(-100) (f32 denormal, and 0 on hardware with FTZ),
matching the reference to absolute error < 2e-27.

The kernel computes the carry chain in f32 (exact for byte values), builds
the nibble one-hot-ish vectors with iota + is_equal, and expands each
token's 1024-float output row with a single broadcast tensor_tensor
multiply. It is output-bandwidth bound: each core writes 32 MiB.

Sharding: pure data parallel over the batch dim, 8192 tokens per core.
"""

import numpy as np

N_CORES = 8
B_FULL = 65536
B_SHARD = B_FULL // N_CORES      # 8192 tokens per core
P = 128                          # SBUF partitions
NPT = B_SHARD // P               # tokens per partition (64)
TPB = 4                          # tokens per partition per output tile
NTILES = NPT // TPB              # output tiles per core (16)

E50 = float(np.float32(np.exp(np.float64(-50.0))))   # 1.9287499e-22


def _emit(tc, nc, a_ap, b_ap, out_ap, npt=NPT, tpb=TPB):
    """Emit the per-core Tile program.

    a_ap, b_ap: [P*npt, 4] int32 DRAM.  out_ap: [P*npt, 1024] f32 DRAM.
    Token t = p*npt + n lives on partition p, free slot n.
    """
    from contextlib import ExitStack
    import concourse.mybir as mybir

    # ramp-up schedule: small first tiles get the store pipeline going
    # early, then steady-state tiles of tpb tokens/partition
    sched = []
    for cand in (1, 1, 2):
        if sum(sched) + cand <= npt and cand < tpb:
            sched.append(cand)
    while sum(sched) < npt:
        sched.append(min(tpb, npt - sum(sched)))

    nc4 = npt * 4                    # free size of per-token-byte tensors
    f32 = mybir.dt.float32
    i32 = mybir.dt.int32
    Alu = mybir.AluOpType

    with ExitStack() as ctx:
        const = ctx.enter_context(tc.tile_pool(name="const", bufs=1))
        pre = ctx.enter_context(tc.tile_pool(name="pre", bufs=1))
        uvs = ctx.enter_context(tc.tile_pool(name="uvs", bufs=3))
        outs = ctx.enter_context(tc.tile_pool(name="outs", bufs=6))

        # --- constants: J[p, m*16+j] = j  (j pattern repeating every 16)
        ji = const.tile([P, 256], i32, tag="ji")
        nc.gpsimd.iota(ji[:], pattern=[[0, 16], [1, 16]], base=0,
                       channel_multiplier=0)
        e50b = const.tile([P, 1], f32, tag="e50b")   # ACT bias vector
        nc.vector.memset(e50b[:], E50)

        # --- load inputs: partition p holds tokens p*npt .. p*npt+npt-1
        ai = pre.tile([P, nc4], i32, tag="ai")
        bi = pre.tile([P, nc4], i32, tag="bi")
        nc.sync.dma_start(ai[:], a_ap.rearrange("(p n) c -> p (n c)", p=P))
        nc.sync.dma_start(bi[:], b_ap.rearrange("(p n) c -> p (n c)", p=P))

        # --- s[p, n, i] = a byte + b byte, then ripple carry in place:
        #     s[:,:,i+1] += (s[:,:,i] >= 256)   (carry propagate, 1 op/byte)
        s = pre.tile([P, nc4], i32, tag="s")
        nc.vector.tensor_add(s[:], ai[:], bi[:])
        s3 = s[:].rearrange("p (n c) -> p n c", c=4)
        for i in range(3):
            nc.vector.scalar_tensor_tensor(
                s3[:, :, i + 1], s3[:, :, i], 256, s3[:, :, i + 1],
                Alu.is_ge, Alu.add)

        # --- nibbles: lo = s & 15, hi = (s >> 4) & 15 (strips carry bits)
        lo = pre.tile([P, nc4], i32, tag="lo")
        hi = pre.tile([P, nc4], i32, tag="hi")
        nc.vector.tensor_scalar(lo[:], s[:], 15, None, Alu.bitwise_and)
        nc.vector.tensor_scalar(hi[:], s[:], 4, 15,
                                Alu.logical_shift_right, Alu.bitwise_and)

        # --- per output tile: tw tokens per partition -> m = tw*4 combos
        out_v = out_ap.rearrange("(p n) f -> p n f", p=P)
        n0 = 0
        for tw in sched:
            m = tw * 4               # (token, byte) combos this tile
            hs = hi[:, n0 * 4:(n0 + tw) * 4]
            ls = lo[:, n0 * 4:(n0 + tw) * 4]

            uh = uvs.tile([P, m * 16], f32, tag="uh")
            ul = uvs.tile([P, m * 16], f32, tag="ul")
            nc.vector.tensor_tensor(
                uh[:].rearrange("p (m j) -> p m j", m=m),
                ji[:, :m * 16].rearrange("p (m j) -> p m j", m=m),
                hs.to_broadcast((P, m, 16)), Alu.is_equal)
            nc.vector.tensor_tensor(
                ul[:].rearrange("p (m k) -> p m k", m=m),
                ji[:, :m * 16].rearrange("p (m k) -> p m k", m=m),
                ls.to_broadcast((P, m, 16)), Alu.is_equal)
            uhe = uvs.tile([P, m * 16], f32, tag="uhe")
            ule = uvs.tile([P, m * 16], f32, tag="ule")
            nc.scalar.add(uhe[:], uh[:], e50b[:])
            nc.scalar.add(ule[:], ul[:], e50b[:])

            ot = outs.tile([P, m * 256], f32, tag="ot")
            nc.vector.tensor_tensor(
                ot[:].rearrange("p (m j k) -> p m j k", m=m, j=16),
                uhe[:].rearrange("p (m j) -> p m j", m=m).to_broadcast(
                    (P, m, 16, 16)),
                ule[:].rearrange("p (m k) -> p m k", m=m).unsqueeze(
                    2).broadcast_to((P, m, 16, 16)),
                Alu.mult)
            nc.sync.dma_start(out_v[:, n0:n0 + tw, :],
                              ot[:].rearrange("p (n f) -> p n f", n=tw))
            n0 += tw


def build_nc(b_shard=B_SHARD, tpb=TPB):
    import concourse.tile as tile
    from concourse import bacc, mybir

    npt = b_shard // P
    nc = bacc.Bacc("TRN2", target_bir_lowering=False, debug=False,
                   num_devices=N_CORES)
    a = nc.dram_tensor("a_idx", [b_shard, 4], mybir.dt.int32,
                       kind="ExternalInput")
    b = nc.dram_tensor("b_idx", [b_shard, 4], mybir.dt.int32,
                       kind="ExternalInput")
    out = nc.dram_tensor("out", [b_shard, 1024], mybir.dt.float32,
                         kind="ExternalOutput")
    with tile.TileContext(nc) as tc:
        _emit(tc, nc, a.ap(), b.ap(), out.ap(), npt=npt, tpb=tpb)
    nc.compile()
    return nc


_NC_CACHE = {}
LAST_RESULTS = None   # BassKernelResults of the most recent kernel() call


def kernel(**inputs):
    a_idx = np.ascontiguousarray(inputs["a_idx"], dtype=np.int32)
    b_idx = np.ascontiguousarray(inputs["b_idx"], dtype=np.int32)
    assert a_idx.shape == (B_FULL, 4) and b_idx.shape == (B_FULL, 4)

    from concourse.bass_utils import run_bass_kernel_spmd

    if "nc" not in _NC_CACHE:
        _NC_CACHE["nc"] = build_nc()
    nc = _NC_CACHE["nc"]

    in_maps = [
        {"a_idx": a_idx[i * B_SHARD:(i + 1) * B_SHARD],
         "b_idx": b_idx[i * B_SHARD:(i + 1) * B_SHARD]}
        for i in range(N_CORES)
    ]
    res = run_bass_kernel_spmd(nc, in_maps, list(range(N_CORES)))
    global LAST_RESULTS
    LAST_RESULTS = res
    out = np.concatenate(
        [r["out"].reshape(B_SHARD, 4, 256) for r in res.results], axis=0)
    return out
